# revision 44
# baseline (speedup 1.0000x reference)
"""Binarized ResNet BasicBlock (2x binarized 3x3 conv + batchnorm + hardtanh,
residual) on 8 Trainium2 NeuronCores, data-parallel over batch.

Math (per reference):
  s1  = conv3x3(sign(x), sign(W1), pad=1)          # integer-valued
  h   = clip(bn1(s1), -1, 1)                       # only sign(h) is consumed
  s2p = conv3x3(sign(h), sign(W2), pad=1) + x
  out = clip(bn2(s2p), -1, 1)

Key points:
  - sign(h) = sign(a1*s1 + c1) per channel (a1 = g1*rsqrt(v1+eps),
    c1 = b1 - m1*a1), so h is never materialized.
  - batchnorm needs global batch stats: each core computes per-channel
    (E[x], E[x^2]) partials over its 4 images; a tiny AllReduce (128x6 f32)
    combines them (equal pixel counts per core, so mean-of-means works).
  - fp8: +/-1 activations/weights in fp8e4 are exact; the 3x3 conv's 27
    (channel-chunk, tap) units are packed into 13 DoubleRow K=256 matmuls +
    1 normal K=128 matmul per output tile.
  - Seam-free plane layout: per input-channel chunk cc there are 3 planes
    (58 rows x 56 cols, stride 3248 = 16B-aligned): A (padded cols 0..55),
    B (cols 1..56 = the real columns), C (cols 2..57). The ACT sign writes
    land in B; A and C are 1-col-shifted SBUF DMA copies. Conv rhs runs are
    then 8 rows x 56 = 448 contiguous cols with no seam (the old padded
    layout burned 464-col runs, +3.6%% matmul time, and needed seam strips
    on evacuation).
  - DoubleRow pair base addresses must be 2B-aligned and pair strides
    16B-aligned. Plane stride 3248 and 2-row stride 112 both qualify, so
    the 27 (cc, dy, dx) taps pack as: 9 (A,B) pairs (dx=0,1 same cc,dy),
    3 (C0,C1) pairs (cc=0,1 same dy, dx=2), 1 (C2@dy0, C2@dy2) pair via a
    custom overlapping AP with pair stride 112, and 1 single (C2@dy1).
  - s1 and s2p stay resident in SBUF as fp16 (integers < 2048: exact; s2p
    adds the fp32 residual, fp16 rounding ~5e-4 relative).
  - AllReduce staging copies ride HWDGE (sync ring), not gpsimd/SWDGE: the
    Q7 descriptor-gen latency sat directly on the serial stats path.
  - bn aggregation per pc is emitted eagerly (right after the last image's
    stats for that pc) so only the last pc's aggregation trails the convs.
"""

import contextlib

import numpy as np
import ml_dtypes

import concourse.bass as bass
import concourse.tile as tile
from concourse import bacc, mybir
from concourse.bass_types import AP
from concourse.bass_utils import run_bass_kernel_spmd
from concourse.replica_groups import maybe_share_collective_output_space

F32 = mybir.dt.float32
F16 = mybir.dt.float16
F8 = mybir.dt.float8e4
F8NP = mybir.dt.np(F8)

NCORES = 8
B, C, H, W = 32, 384, 56, 56
P = C
BPC = B // NCORES         # images per core
NCC = C // 128            # input channel chunks
NPC = P // 128            # output channel chunks
HP = H + 2                # padded rows
NPIX = H * W              # 3136
CHUNK_ROWS = 8            # output rows per PSUM tile
NCHUNK = H // CHUNK_ROWS  # 7
CHW = CHUNK_ROWS * W      # 448
EPS = 1e-5

CSTRIDE = HP * W          # 3248 fp8 plane stride (58 rows x 56 cols), 16B mult
RUN = CHUNK_ROWS * W      # 448: contiguous seam-free rhs run
NPLANE = 9                # A0 B0 A1 B1 A2 B2 C0 C1 C2
XIN_BUFS = 6              # xin/xr/oc staging depth

# fp8 unit schedule: 13 DoubleRow pairs + 1 single cover the 27 (cc, dy, dx)
# conv units. Planes (58x56 each): A-cc at 2cc (padded cols 0..55), B-cc at
# 2cc+1 (cols 1..56), C-cc at 6+cc (cols 2..57).
#  dx01 pair (cc, dy): taps (cc,dy,0)@A-cc, (cc,dy,1)@B-cc;
#    rhs sx[:, 2cc:2cc+2, q:q+RUN], q=(y0+dy)*W
#  cc01 pair (dy): taps (0,dy,2)@C0, (1,dy,2)@C1;
#    rhs sx[:, 6:8, q:q+RUN], q=(y0+dy)*W
#  xp pair: taps (2,0,2), (2,2,2) both @C2, pair stride 2 rows = 112 bytes;
#    custom AP at q=y0*W
#  single: tap (2,1,2)@C2; rhs sx[:, 8, q:q+RUN], q=(y0+1)*W
FP8_PAIRS = (
    [("dx01", cc, dy) for dy in range(3) for cc in range(3)]
    + [("cc01", None, dy) for dy in range(3)]
    + [("xp", None, None)]
)
NUNIT_FP8 = len(FP8_PAIRS) + 1  # 14


def _fp8_pair_units():
    """(uA, uB) tap indices per FP8_PAIRS entry; each tap is (cc, dy, dx)."""
    out = []
    for kind, cc, dy in FP8_PAIRS:
        if kind == "dx01":
            out.append(((cc, dy, 0), (cc, dy, 1)))
        elif kind == "cc01":
            out.append(((0, dy, 2), (1, dy, 2)))
        else:  # xp: C2 rows dy=0 and dy=2
            out.append(((2, 0, 2), (2, 2, 2)))
    return out


FP8_SINGLE_UNIT = (2, 1, 2)


def _prep_weight_fp8(w):
    """[P, C, 3, 3] -> (pairs [128, 13*NPC*256], single [128, NPC*128]) fp8
    sign values."""
    ws = np.sign(w.astype(np.float32))
    arr = ws.transpose(1, 2, 3, 0).reshape(NCC, 128, 3, 3, NPC, 128)

    def unit(cc, dy, dx):  # [128 (c), NPC, 128 (m)]
        return arr[cc, :, dy, dx]

    npair = len(FP8_PAIRS)
    wp = np.zeros((128, npair, NPC, 2, 128), np.float32)
    for j, (uA, uB) in enumerate(_fp8_pair_units()):
        wp[:, j, :, 0] = unit(*uA)
        wp[:, j, :, 1] = unit(*uB)
    wsg = unit(*FP8_SINGLE_UNIT)  # [128, NPC, 128]
    return (
        np.ascontiguousarray(wp.reshape(128, -1)).astype(F8NP),
        np.ascontiguousarray(wsg.reshape(128, -1)).astype(F8NP),
    )


def _prep_vecs(g1, b1, g2, b2):
    """-> [128, NPC, 4] f32: per-partition (p_in) per-chunk (pc) gamma/beta."""
    out = np.empty((128, NPC, 4), np.float32)
    for k, v in enumerate((g1, b1, g2, b2)):
        out[:, :, k] = v.astype(np.float32).reshape(NPC, 128).T
    return out


def _stats_to_scale_bias(nc, singles, allout, vecs_sb, eps_tile, gk, bk, name,
                         ncores):
    """allout [128, 1, 2] summed (E, E2) over cores for ONE pc chunk ->
    a, c [128, 1, 1]. vecs_sb is the [128, 4] slice for this pc."""
    Eg = singles.tile([128, 1, 1], F32, name=f"{name}_Eg")
    E2g = singles.tile([128, 1, 1], F32, name=f"{name}_E2g")
    var = singles.tile([128, 1, 1], F32, name=f"{name}_var")
    tmp = singles.tile([128, 1, 1], F32, name=f"{name}_tmp")
    sd = singles.tile([128, 1, 1], F32, name=f"{name}_sd")
    rs = singles.tile([128, 1, 1], F32, name=f"{name}_rs")
    a = singles.tile([128, 1, 1], F32, name=f"{name}_a")
    c = singles.tile([128, 1, 1], F32, name=f"{name}_c")
    nc.scalar.mul(Eg[:], allout[:, :, 0:1], 1.0 / ncores)
    nc.scalar.mul(E2g[:], allout[:, :, 1:2], 1.0 / ncores)
    nc.vector.tensor_mul(tmp[:], Eg[:], Eg[:])
    nc.vector.tensor_tensor(
        out=var[:], in0=E2g[:], in1=tmp[:], op=mybir.AluOpType.subtract
    )
    nc.scalar.activation(
        sd[:], var[:], mybir.ActivationFunctionType.Sqrt, bias=eps_tile[:],
        scale=1.0,
    )
    nc.vector.reciprocal(out=rs[:], in_=sd[:])
    nc.vector.tensor_mul(a[:], rs[:], vecs_sb[:, gk : gk + 1])
    nc.vector.tensor_mul(tmp[:], Eg[:], a[:])
    nc.vector.tensor_tensor(
        out=c[:], in0=vecs_sb[:, bk : bk + 1], in1=tmp[:],
        op=mybir.AluOpType.subtract,
    )
    return a, c


def _emit_conv_fp8(nc, psum_pool, wp_view, ws_view, sx_tile, pc):
    """Weight-stationary fp8 DoubleRow conv for one (img, pc): returns NCHUNK
    psum tiles [128, RUN]. All 7 chunks accumulate in one weight-stationary
    pass (7 of 8 PSUM banks)."""
    perf = mybir.MatmulPerfMode.DoubleRow
    c2 = sx_tile[:, 8, :]  # C2 plane [128, CSTRIDE]
    c2_part = list(c2.ap[0])
    pss = {}
    for chunk in range(NCHUNK):
        pss[chunk] = psum_pool.tile([128, RUN], F32, name="ps", tag="ps")
    u = 0
    for j, (kind, cc, dy) in enumerate(FP8_PAIRS):
        lhsT = wp_view[:, j, pc]
        for chunk in range(NCHUNK):
            y0 = chunk * CHUNK_ROWS
            if kind == "dx01":
                q = (y0 + dy) * W
                rhs = sx_tile[:, 2 * cc : 2 * cc + 2, q : q + RUN]
            elif kind == "cc01":
                q = (y0 + dy) * W
                rhs = sx_tile[:, 6:8, q : q + RUN]
            else:  # xp: C2 @ dy0 paired with C2 @ dy2 (pair stride 112B)
                rhs = AP(c2.tensor, c2.offset + y0 * W,
                         [c2_part, [2 * W, 2], [1, RUN]])
            nc.tensor.matmul(
                pss[chunk][:], lhsT, rhs,
                start=(u == 0), stop=(u == NUNIT_FP8 - 1), perf_mode=perf,
            )
        u += 1
    lhsT = ws_view[:, pc]
    for chunk in range(NCHUNK):
        y0 = chunk * CHUNK_ROWS
        q = (y0 + 1) * W
        rhs = sx_tile[:, 8, q : q + RUN]
        nc.tensor.matmul(
            pss[chunk][:], lhsT, rhs,
            start=(u == 0), stop=(u == NUNIT_FP8 - 1),
        )
    return [pss[c] for c in range(NCHUNK)]


# half split for plane building and x staging: chunks 0-3 cover B rows
# 1..32, chunks 4-6 cover rows 33..56.
HALF_CHUNKS = (range(0, 4), range(4, NCHUNK))
HALF_ROWS = ((1, 33), (33, 57))
HALF_PIX = 4 * CHW          # 1792: staging tile size (half 0; half 1 = 1344)
HALF_NPIX = (4 * CHW, 3 * CHW)
# image 0's prep is on the critical path (nothing to hide it under), so it
# runs at quarter granularity; later images prep under the previous image's
# conv shadow at half granularity
PREP_SPLITS_FIRST = ((1, 15), (15, 29), (29, 43), (43, 57))
PREP_SPLITS_REST = HALF_ROWS


def prep_splits(img):
    return PREP_SPLITS_FIRST if img == 0 else PREP_SPLITS_REST


def build_program(bpc=BPC, ncores=NCORES, timing_iters=None):
    nc = bacc.Bacc(
        "TRN2",
        target_bir_lowering=False,
        debug=False,
        enable_asserts=True,
        num_devices=ncores,
    )
    x_d = nc.dram_tensor("x", [bpc, C, H, W], F32, kind="ExternalInput").ap()
    wpair_elems = len(FP8_PAIRS) * NPC * 256
    w1p_d = nc.dram_tensor("w1p", [128, wpair_elems], F8,
                           kind="ExternalInput").ap()
    w1s_d = nc.dram_tensor("w1s", [128, NPC * 128], F8,
                           kind="ExternalInput").ap()
    w2p_d = nc.dram_tensor("w2p", [128, wpair_elems], F8,
                           kind="ExternalInput").ap()
    w2s_d = nc.dram_tensor("w2s", [128, NPC * 128], F8,
                           kind="ExternalInput").ap()
    vecs_d = nc.dram_tensor("vecs", [128, NPC, 4], F32,
                            kind="ExternalInput").ap()
    out_d = nc.dram_tensor("out", [bpc, C, H, W], F32,
                           kind="ExternalOutput").ap()

    with tile.TileContext(nc) as tc:
        with (
            tc.tile_pool(name="weights", bufs=2) as wpool,
            tc.tile_pool(name="singles", bufs=1) as singles,
            tc.tile_pool(name="sx", bufs=1) as sxpool,
            tc.tile_pool(name="acc", bufs=3 * bpc) as accpool,
            tc.tile_pool(name="stage", bufs=4) as stagepool,
            tc.tile_pool(name="oc", bufs=XIN_BUFS) as ocpool,
            tc.tile_pool(name="stats", bufs=1) as stpool,
            tc.tile_pool(name="psum", bufs=8, space="PSUM") as psum_pool,
            tc.tile_pool(name="dram", bufs=1, space="DRAM") as dram,
        ):
            # ---- constants (outside the timing loop) ----
            # weights ride the scalar (ACT) HWDGE ring; the real build emits
            # their loads mid-prep of image 0 so the serial DMA engine mover
            # serves the first xin halves first (w1 is only needed by the
            # first matmul ~12us in, w2 only by pass B)
            w1p_sb = wpool.tile([128, wpair_elems], F8, name="w1p_sb",
                                tag="wp")
            w1s_sb = wpool.tile([128, NPC * 128], F8, name="w1s_sb", tag="ws")
            w2p_sb = wpool.tile([128, wpair_elems], F8, name="w2p_sb",
                                tag="wp")
            w2s_sb = wpool.tile([128, NPC * 128], F8, name="w2s_sb", tag="ws")

            # w1 on the scalar HWDGE ring (needed by the first matmul ~7us
            # in); w2 via the idle gpsimd/SWDGE ring so its 10KB doesn't
            # occupy the DMA mover while image 0's xin quarters stream
            # (pass B is ~150us away, SWDGE's slow descriptor gen is fine)
            nc.scalar.dma_start(out=w1p_sb, in_=w1p_d)
            nc.scalar.dma_start(out=w1s_sb, in_=w1s_d)
            nc.gpsimd.dma_start(out=w2p_sb, in_=w2p_d)
            nc.gpsimd.dma_start(out=w2s_sb, in_=w2s_d)
            w1p_v = w1p_sb.rearrange("p (j q i m) -> p j q i m",
                                     j=len(FP8_PAIRS), q=NPC, i=2)
            w2p_v = w2p_sb.rearrange("p (j q i m) -> p j q i m",
                                     j=len(FP8_PAIRS), q=NPC, i=2)
            w1s_v = w1s_sb.rearrange("p (q m) -> p q m", q=NPC)
            w2s_v = w2s_sb.rearrange("p (q m) -> p q m", q=NPC)
            vecs_sb = singles.tile([128, NPC, 4], F32)
            nc.sync.dma_start(out=vecs_sb, in_=vecs_d)
            eps_tile = singles.tile([128, 1], F32)
            nc.vector.memset(eps_tile, EPS)

            # persistent sign planes. Only the pad rows 0 and 57 need the
            # initial clear (data rows 1..56 are fully written per image:
            # signs cover B, shifted copies + wrap-fix memsets cover A/C),
            # so the init memsets touch just 2 rows per plane.
            sxt = []
            for s in range(2):
                t = sxpool.tile([128, NPLANE, CSTRIDE], F8, name=f"sx{s}")
                for pl in range(NPLANE):
                    v = t[:, pl, :].rearrange("p (h w) -> p h w", w=W)
                    eng = (nc.vector, nc.gpsimd)[pl % 2]
                    eng.memset(v[:, 0 : HP : HP - 1, :], 0.0)
                sxt.append(t)

            bnst1 = [
                stpool.tile([128, bpc * NCHUNK, 6], F32, name=f"bnst1_{pc}")
                for pc in range(NPC)
            ]
            bnst2 = [
                stpool.tile([128, bpc * NCHUNK, 6], F32, name=f"bnst2_{pc}")
                for pc in range(NPC)
            ]

            cc_addr_space = (
                "Local" if timing_iters is not None
                else maybe_share_collective_output_space(
                    "AllReduce", [list(range(ncores))]
                )
            )

            def do_allreduce(cin, cout):
                if timing_iters is None:
                    nc.gpsimd.collective_compute(
                        "AllReduce",
                        mybir.AluOpType.add,
                        replica_groups=[list(range(ncores))],
                        ins=[cin.opt()],
                        outs=[cout.opt()],
                    )
                else:
                    nc.sync.dma_start(out=cout, in_=cin)

            def make_plane_copies(sx_tile, rows, grp=None):
                """A = B shifted right 1 col, C = B shifted left 1 col, for
                the given row range. One contiguous 1-byte-shifted DMA per
                direction spans the group's cc planes (strided over the
                plane dim); the per-row wrap garbage (A col 0 picks up
                B[r-1,55], C col 55 picks up B[r+1,0]) is re-zeroed with two
                small strided memsets. grp "01"/"2" limits to those cc
                planes (pass B: pc2's threshold arrives last)."""
                r0, r1 = rows
                if grp == "01":
                    a_sl, b_sl, c_sl = slice(0, 3, 2), slice(1, 4, 2), \
                        slice(6, 8)
                elif grp == "2":
                    a_sl, b_sl, c_sl = slice(4, 5), slice(5, 6), slice(8, 9)
                else:
                    a_sl, b_sl, c_sl = slice(0, 5, 2), slice(1, 6, 2), \
                        slice(6, 9)
                nc.scalar.dma_start(
                    out=sx_tile[:, a_sl, r0 * W + 1 : r1 * W],
                    in_=sx_tile[:, b_sl, r0 * W : r1 * W - 1])
                nc.sync.dma_start(
                    out=sx_tile[:, c_sl, r0 * W : r1 * W - 1],
                    in_=sx_tile[:, b_sl, r0 * W + 1 : r1 * W])
                a_v = sx_tile[:, a_sl, :].rearrange(
                    "p a (h w) -> p a h w", w=W)
                c_v = sx_tile[:, c_sl, :].rearrange(
                    "p a (h w) -> p a h w", w=W)
                nc.gpsimd.memset(a_v[:, :, r0:r1, 0:1], 0.0)
                nc.gpsimd.memset(c_v[:, :, r0:r1, W - 1 : W], 0.0)

            def emit_bn_chain(pc, bnst, tag, gk, bk):
                """Per-pc tail of a conv pass: aggregate this pc's stats,
                stage to DRAM, AllReduce (its own tiny collective so pc0/pc1
                complete while later convs still run), read back, and
                compute the (a, c) scale/bias. Returns (a, c) [128, 1, 1]."""
                allin = singles.tile([128, 2], F32, name=f"allin{tag}_{pc}")
                mv = stpool.tile([128, 2], F32, name=f"mv{tag}_{pc}")
                nc.vector.bn_aggr(out=mv, in_=bnst[pc])
                nc.vector.tensor_copy(allin[:, 0:1], mv[:, 0:1])
                sq = stpool.tile([128, 1], F32, name=f"sq{tag}_{pc}")
                nc.vector.tensor_mul(sq, mv[:, 0:1], mv[:, 0:1])
                nc.vector.tensor_tensor(
                    out=allin[:, 1:2], in0=mv[:, 1:2], in1=sq,
                    op=mybir.AluOpType.add,
                )
                cin = dram.tile([128, 2], F32, name=f"cc{tag}_{pc}_in")
                cout = dram.tile([128, 2], F32, name=f"cc{tag}_{pc}_out",
                                 addr_space=cc_addr_space)
                nc.sync.dma_start(out=cin, in_=allin)
                do_allreduce(cin, cout)
                allout = singles.tile([128, 1, 2], F32,
                                      name=f"allout{tag}_{pc}")
                nc.sync.dma_start(
                    out=allout.rearrange("p a b -> p (a b)"), in_=cout)
                return _stats_to_scale_bias(
                    nc, singles, allout, vecs_sb[:, pc], eps_tile, gk, bk,
                    f"bn{tag}_{pc}", ncores,
                )

            loop_cm = (tc.For_i(0, timing_iters, 1) if timing_iters
                       else contextlib.nullcontext())
            with loop_cm:
                # ---- pass A: conv1, stats, s1 resident in fp16 ----
                s1 = {}
                s2 = {}
                a1 = [None] * NPC
                c1 = [None] * NPC
                a2 = [None] * NPC
                c2 = [None] * NPC
                for img in range(bpc):
                    sx_tile = sxt[img % 2]
                    for rows in prep_splits(img):
                        r0, r1 = rows
                        npix_h = (r1 - r0) * W
                        for cc in range(NCC):
                            xin = stagepool.tile([128, HALF_PIX], F32,
                                               name="xin", tag="stage")
                            nc.sync.dma_start(
                                out=xin[:, 0:npix_h],
                                in_=x_d[img, cc * 128 : (cc + 1) * 128,
                                        r0 - 1 : r1 - 1],
                            )
                            dst = sx_tile[:, 2 * cc + 1, r0 * W : r1 * W]
                            nc.scalar.activation(
                                dst, xin[:, 0:npix_h],
                                mybir.ActivationFunctionType.Sign,
                            )
                        make_plane_copies(sx_tile, rows)
                    for pc in range(NPC):
                        s1t = accpool.tile([128, NPIX], F16,
                                           name=f"s1_{img}_{pc}", tag="acc")
                        s1[(img, pc)] = s1t
                        pss = _emit_conv_fp8(nc, psum_pool, w1p_v, w1s_v,
                                             sx_tile, pc)
                        last = img == bpc - 1 and pc == NPC - 1
                        for chunk in range(NCHUNK):
                            sl = slice(chunk * CHW, (chunk + 1) * CHW)
                            nc.scalar.copy(s1t[:, sl], pss[chunk][:])
                            # the final (img, pc) gates the bn1 pc2
                            # AllReduce: read stats straight off PSUM so
                            # they don't chain behind the ACT evacuation
                            nc.vector.bn_stats(
                                out=bnst1[pc][:, img * NCHUNK + chunk, :],
                                in_=pss[chunk][:] if last else s1t[:, sl],
                            )
                        if img == bpc - 1:
                            # bn1 chain per pc: pc0/pc1's AllReduce flies
                            # while pc1/pc2 convs still run
                            a1[pc], c1[pc] = emit_bn_chain(
                                pc, bnst1, "1", 0, 1)

                def emit_pass_c(pc, ring=None):
                    """scale/bias + clip + store for one output-channel
                    chunk, streaming per (img, chunk) across ACT/DVE/Pool
                    with the store DMA right behind each clip. ring picks
                    the HWDGE ring for stores (early-emitted pc0 rides the
                    scalar ring so queued store triggers don't block later
                    xr loads on the sync ring)."""
                    ring = ring or nc.sync
                    for img in range(bpc):
                        s2t = s2[(img, pc)]
                        for chunk in range(NCHUNK):
                            sl = slice(chunk * CHW, (chunk + 1) * CHW)
                            oc = ocpool.tile([128, CHW], F32, name="oc",
                                             tag="oc")
                            if chunk % 2 == 0:
                                nc.scalar.activation(
                                    oc[:], s2t[:, sl],
                                    mybir.ActivationFunctionType.Identity,
                                    bias=c2[pc][:, 0, :],
                                    scale=a2[pc][:, 0, :],
                                )
                                nc.vector.tensor_scalar(
                                    out=oc[:], in0=oc[:], scalar1=1.0,
                                    scalar2=-1.0, op0=mybir.AluOpType.min,
                                    op1=mybir.AluOpType.max,
                                )
                            else:
                                nc.vector.tensor_scalar(
                                    out=oc[:], in0=s2t[:, sl],
                                    scalar1=a2[pc][:, 0, :],
                                    scalar2=c2[pc][:, 0, :],
                                    op0=mybir.AluOpType.mult,
                                    op1=mybir.AluOpType.add,
                                )
                                nc.gpsimd.tensor_scalar(
                                    out=oc[:], in0=oc[:], scalar1=1.0,
                                    scalar2=-1.0, op0=mybir.AluOpType.min,
                                    op1=mybir.AluOpType.max,
                                )
                            y0 = chunk * CHUNK_ROWS
                            ring.dma_start(
                                out=out_d[img, pc * 128 : (pc + 1) * 128,
                                          y0 : y0 + CHUNK_ROWS],
                                in_=oc.rearrange("p (h w) -> p h w", w=W),
                            )

                # ---- pass B: sign threshold, conv2 + residual, stats ----
                xr_halves = {}
                for img in range(bpc):
                    sh_tile = sxt[img % 2]

                    def thresh_sign(pc, rows):
                        r0, r1 = rows
                        dst = sh_tile[:, 2 * pc + 1, r0 * W : r1 * W]
                        src = s1[(img, pc)][:, (r0 - 1) * W : (r1 - 1) * W]
                        nc.scalar.activation(
                            dst, src, mybir.ActivationFunctionType.Sign,
                            bias=c1[pc][:, 0, :], scale=a1[pc][:, 0, :],
                        )

                    # pc0/pc1 thresholds arrive first (per-pc AllReduce), so
                    # their signs + copies go ahead; pc2 trails
                    for rows in prep_splits(img):
                        for pc in (0, 1):
                            thresh_sign(pc, rows)
                    for rows in prep_splits(img):
                        make_plane_copies(sh_tile, rows, grp="01")
                    for rows in prep_splits(img):
                        thresh_sign(2, rows)
                        make_plane_copies(sh_tile, rows, grp="2")
                    for pc in range(NPC):
                        # residual halves prefetched at pc-block start: they
                        # ride the conv shadow, the first TT lands ~13us in
                        for half in range(2):
                            r0, r1 = HALF_ROWS[half]
                            xr = stagepool.tile([128, HALF_PIX], F32, name="xr",
                                             tag="stage")
                            nc.sync.dma_start(
                                out=xr[:, 0 : HALF_NPIX[half]],
                                in_=x_d[img, pc * 128 : (pc + 1) * 128,
                                        r0 - 1 : r1 - 1],
                            )
                            xr_halves[(img, pc, half)] = xr
                        s2t = accpool.tile([128, NPIX], F16,
                                           name=f"s2_{img}_{pc}", tag="acc")
                        s2[(img, pc)] = s2t
                        pss = _emit_conv_fp8(nc, psum_pool, w2p_v, w2s_v,
                                             sh_tile, pc)
                        if img == bpc - 1 and pc == NPC - 1:
                            # pc0's output pass slots in here: its deps
                            # (a2[0] via its already-flying AllReduce, all
                            # s2[*,0]) land while these pc1 convs run, so
                            # pc0's clips+stores overlap the pc1+pc2 convs
                            # and the bn2 pc2 AllReduce
                            emit_pass_c(0)
                        for chunk in range(NCHUNK):
                            half = 0 if chunk < 4 else 1
                            xr = xr_halves[(img, pc, half)]
                            xsl = slice(chunk * CHW - half * HALF_PIX,
                                        (chunk + 1) * CHW - half * HALF_PIX)
                            sl = slice(chunk * CHW, (chunk + 1) * CHW)
                            nc.vector.tensor_tensor(
                                out=s2t[:, sl], in0=pss[chunk][:],
                                in1=xr[:, xsl],
                                op=mybir.AluOpType.add,
                            )
                            nc.vector.bn_stats(
                                out=bnst2[pc][:, img * NCHUNK + chunk, :],
                                in_=s2t[:, sl],
                            )
                        if img == bpc - 1:
                            a2[pc], c2[pc] = emit_bn_chain(
                                pc, bnst2, "2", 2, 3)

                # ---- pass C: pc1/pc2 (pc0 was emitted inside the last
                # image's pc2 conv block so its stores overlap those convs
                # and the bn2 pc2 AllReduce) ----
                emit_pass_c(1)
                emit_pass_c(2)

    nc.compile()
    return nc


_PROGRAM = None


def _get_program():
    global _PROGRAM
    if _PROGRAM is None:
        _PROGRAM = build_program()
    return _PROGRAM


def make_in_maps(x, W1, W2, g1, b1, g2, b2, bpc=BPC, ncores=NCORES):
    vecs = _prep_vecs(np.asarray(g1), np.asarray(b1), np.asarray(g2),
                      np.asarray(b2))
    x = np.ascontiguousarray(np.asarray(x, dtype=np.float32))
    w1p, w1s = _prep_weight_fp8(np.asarray(W1))
    w2p, w2s = _prep_weight_fp8(np.asarray(W2))
    wmap = {"w1p": w1p, "w1s": w1s, "w2p": w2p, "w2s": w2s}
    return [
        {"x": x[core * bpc : (core + 1) * bpc], "vecs": vecs, **wmap}
        for core in range(ncores)
    ]


def kernel(x, W1, W2, g1, b1, g2, b2, trace=False):
    nc = _get_program()
    in_maps = make_in_maps(x, W1, W2, g1, b1, g2, b2)
    res = run_bass_kernel_spmd(
        nc, in_maps, core_ids=list(range(NCORES)), trace=trace
    )
    out = np.concatenate([res.results[c]["out"] for c in range(NCORES)], axis=0)
    kernel.last_results = res
    return out
